# revision 14
# baseline (speedup 1.0000x reference)
"""LocalGatedPropagation Trainium2 Bass kernel (8 NeuronCores).

Sharding: core k -> batch n = k//4, heads {2*(k%4), 2*(k%4)+1}. Each core
computes its 2 heads' windowed attention (15x15 over 30x30), gating and
depthwise 5x5 conv on its 64 gated channels, and a partial 256->128
projection. Host sums the 4 partial projections per batch and reassembles
local_attn from per-core compact shards.

Key device techniques:
  - patch-matmul for windowed correlation: per 4-row band, qk[p, pp] over a
    20-row / 600-px patch (all window offsets become plain SBUF views of a
    vertically zero-padded feature map)
  - exp(rel) scattered into patch layout by gpsimd local_scatter with
    per-partition indices, moving f32 as uint16 pairs (exact)
  - vertically-invalid patch rows killed by two contiguous memsets on G
  - softmax as E = exp(qk/T)*G with fused row-sum (stt accum_out); compact
    local_attn extracted by a second local_scatter (inverse index map)
  - aggregation via PE transposes of E + 5 accumulating matmuls against
    pixel-major silu(Wv@v) tiles
  - depthwise conv as 25 diagonal-weight matmuls on a 34-stride gutter
    layout (flat shifts can't wrap into neighbor rows)
"""
import math
import numpy as np

N, DQK, DVU, NH, MAXD = 2, 256, 256, 8, 7
H = W = 30
WS = 2 * MAXD + 1            # 15
WS2 = WS * WS                # 225
DATT = DQK // NH             # 32
HID = DVU // NH              # 32
DOUT = DVU // 2              # 128
PIX = H * W                  # 900
T = math.sqrt(DATT)
NB = 8                       # 4-row bands (band 7 has 2 rows)
PATCH = 600                  # 20 patch rows * 30 cols
GUT = 34
GBUF = GUT * GUT + 8         # gutter buffer width (pad for tap views)
VPW = 1440                   # f_vpad: 240 zeros | 900 | 300 zeros

_CACHE = {}


def _band_pix(b):
    return 120 if b < 7 else 60


def _build_indices():
    g_idx = np.full((128, 2 * WS2), -1, dtype=np.int16)
    ex_idx = np.full((128, 2 * PATCH), -1, dtype=np.int16)
    for p in range(120):
        r, x = divmod(p, 30)
        for dyy in range(WS):
            for dx in range(WS):
                xx = x + dx - 7
                if not (0 <= xx < 30):
                    continue
                o = dyy * WS + dx
                pp = 30 * (r + dyy + 1) + xx
                g_idx[p, 2 * o] = 2 * pp
                g_idx[p, 2 * o + 1] = 2 * pp + 1
                ex_idx[p, 2 * pp] = 2 * o
                ex_idx[p, 2 * pp + 1] = 2 * o + 1
    return g_idx, ex_idx


def _build_program():
    import concourse.tile as tile
    from concourse import bacc, mybir
    dt = mybir.dt
    f32 = dt.float32
    Alu = mybir.AluOpType
    Act = mybir.ActivationFunctionType

    nc = bacc.Bacc(None, target_bir_lowering=False)

    d_q = nc.dram_tensor("q", [DQK, PIX], f32, kind="ExternalInput")
    d_v = nc.dram_tensor("v", [DVU, PIX], f32, kind="ExternalInput")
    d_u = nc.dram_tensor("u", [DVU, PIX], f32, kind="ExternalInput")
    d_wqkT = nc.dram_tensor("wqkT", [2, 128, 64], f32, kind="ExternalInput")
    d_bqk = nc.dram_tensor("bqk", [64, 1], f32, kind="ExternalInput")
    d_wrelT = nc.dram_tensor("wrelT", [64, WS2], f32, kind="ExternalInput")
    d_brel = nc.dram_tensor("brel", [33, WS2], f32, kind="ExternalInput")
    d_wvT = nc.dram_tensor("wvT", [2, 128, 32], f32, kind="ExternalInput")
    d_bv = nc.dram_tensor("bv", [1, 2, 32], f32, kind="ExternalInput")
    d_wuT = nc.dram_tensor("wuT", [2, 128, 32], f32, kind="ExternalInput")
    d_bu = nc.dram_tensor("bu", [1, 2, 32], f32, kind="ExternalInput")
    d_wdw = nc.dram_tensor("wdw", [64, 25 * 64], f32, kind="ExternalInput")
    d_wproj = nc.dram_tensor("wprojT", [64, DOUT], f32, kind="ExternalInput")
    d_gidx = nc.dram_tensor("g_idx", [128, 2 * WS2], dt.int16, kind="ExternalInput")
    d_exidx = nc.dram_tensor("ex_idx", [128, 2 * PATCH], dt.int16, kind="ExternalInput")
    d_ident = nc.dram_tensor("ident", [128, 128], f32, kind="ExternalInput")
    d_attn = nc.dram_tensor("attn_out", [2, PIX, WS2], f32, kind="ExternalOutput")
    d_proj = nc.dram_tensor("proj_out", [DOUT, PIX], f32, kind="ExternalOutput")

    with tile.TileContext(nc) as tc:
        with (
            tc.tile_pool(name="cst", bufs=1) as cst,
            tc.tile_pool(name="inp", bufs=1) as inp,
            tc.tile_pool(name="bandp", bufs=2) as bandp,
            tc.tile_pool(name="ps1", bufs=1, space="PSUM") as ps1,
            tc.tile_pool(name="psT", bufs=2, space="PSUM") as psT,
        ):
            # ---------- constants ----------
            def cdma(shape, dtt, dram, tag):
                t = cst.tile(shape, dtt, tag=tag)
                nc.sync.dma_start(t[:], dram[:])
                return t

            wqkT0 = cdma([128, 64], f32, d_wqkT[0], "wqk0")
            wqkT1 = cdma([128, 64], f32, d_wqkT[1], "wqk1")
            bqk = cdma([64, 1], f32, d_bqk, "bqk")
            wrelT = cdma([64, WS2], f32, d_wrelT, "wrel")
            brel = cdma([33, WS2], f32, d_brel, "brel")
            wvT0 = cdma([128, 32], f32, d_wvT[0], "wv0")
            wvT1 = cdma([128, 32], f32, d_wvT[1], "wv1")
            wuT0 = cdma([128, 32], f32, d_wuT[0], "wu0")
            wuT1 = cdma([128, 32], f32, d_wuT[1], "wu1")
            bv = cdma([1, 2, 32], f32, d_bv, "bv")
            bu = cdma([1, 2, 32], f32, d_bu, "bu")
            wdw = cdma([64, 25 * 64], f32, d_wdw, "wdw")
            wproj = cdma([64, DOUT], f32, d_wproj, "wproj")
            gidx = cdma([128, 2 * WS2], dt.int16, d_gidx, "gidx")
            exidx = cdma([128, 2 * PATCH], dt.int16, d_exidx, "exidx")
            ident = cdma([128, 128], f32, d_ident, "ident")
            ones2 = cst.tile([33, 512], f32, tag="ones2")
            nc.vector.memset(ones2[0:1, :], 1.0)
            nc.vector.memset(ones2[32:33, :], 1.0)

            # ---------- activations ----------
            q0 = cdma([128, PIX], f32, d_q[0:128, :], "q0")
            q1 = cdma([128, PIX], f32, d_q[128:256, :], "q1")
            v0 = cdma([128, PIX], f32, d_v[0:128, :], "v0")
            v1 = cdma([128, PIX], f32, d_v[128:256, :], "v1")
            u0 = cdma([128, PIX], f32, d_u[0:128, :], "u0")
            u1 = cdma([128, PIX], f32, d_u[128:256, :], "u1")

            # ---------- qk_feat ----------
            f_ps = ps1.tile([128, 1024], f32, tag="ph")
            for nn0, nn1 in ((0, 512), (512, PIX)):
                nc.tensor.matmul(f_ps[0:64, nn0:nn1], wqkT0[:], q0[:, nn0:nn1],
                                 start=True, stop=False)
                nc.tensor.matmul(f_ps[0:64, nn0:nn1], wqkT1[:], q1[:, nn0:nn1],
                                 start=False, stop=True)
            f_vpad = inp.tile([64, VPW], f32, tag="fvpad")
            nc.gpsimd.memset(f_vpad[:, 0:240], 0.0)
            nc.gpsimd.memset(f_vpad[:, 1140:VPW], 0.0)
            for nn0, nn1 in ((0, 512), (512, PIX)):
                nc.vector.tensor_scalar_add(f_vpad[:, 240 + nn0:240 + nn1],
                                            f_ps[0:64, nn0:nn1], bqk[:])
            f_scaled = inp.tile([64, PIX], f32, tag="fscaled")
            nc.vector.tensor_scalar_mul(f_scaled[:], f_vpad[:, 240:1140], 1.0 / T)

            # ---------- vT tiles (pixel-major, head-major cols) ----------
            vT = inp.tile([128, NB, 64], f32, tag="vT")
            nc.gpsimd.memset(vT[:, 7, :], 0.0)
            for tau in range(NB):
                npx = _band_pix(tau)
                psl = tau * 120
                vps = ps1.tile([128, 512], f32, tag="ph")
                for g, (vg, wg) in enumerate(((v0, wvT0), (v1, wvT1))):
                    csl = slice(32 * g, 32 * g + 32)
                    nc.tensor.matmul(vps[0:npx, csl], vg[:, psl:psl + npx],
                                     wg[:], start=True, stop=False)
                    nc.tensor.matmul(vps[0:npx, csl], ones2[0:1, 0:npx],
                                     bv[:, g, :], start=False, stop=True)
                    sg = bandp.tile([128, 32], f32, tag="vsg")
                    nc.scalar.activation(sg[0:npx, :], vps[0:npx, csl], Act.Sigmoid)
                    dst = vT[0:npx, tau, :].rearrange("p (hl c) -> p hl c", hl=2)
                    nc.vector.scalar_tensor_tensor(
                        dst[:, :, 16 * g:16 * g + 16],
                        vps[0:npx, csl].rearrange("p (hl c) -> p hl c", hl=2),
                        1.0,
                        sg[0:npx, :].rearrange("p (hl c) -> p hl c", hl=2),
                        op0=Alu.mult, op1=Alu.mult)

            # ---------- uu (channel-major) then permute to head-major ----------
            ups = ps1.tile([128, 1024], f32, tag="ph")
            for g, (ug, wg) in enumerate(((u0, wuT0), (u1, wuT1))):
                csl = slice(32 * g, 32 * g + 32)
                for nn0, nn1 in ((0, 512), (512, PIX)):
                    nc.tensor.matmul(ups[csl, nn0:nn1], wg[:], ug[:, nn0:nn1],
                                     start=True, stop=False)
                    nc.tensor.matmul(ups[csl, nn0:nn1], bu[:, g, :],
                                     ones2[0:1, 0:nn1 - nn0], start=False, stop=True)
            uu_g = inp.tile([64, PIX], f32, tag="uug")
            for g in range(2):
                csl = slice(32 * g, 32 * g + 32)
                sg = bandp.tile([64, PIX], f32, tag="usg")
                for nn0, nn1 in ((0, 512), (512, PIX)):
                    nc.scalar.activation(sg[csl, nn0:nn1], ups[csl, nn0:nn1],
                                         Act.Sigmoid)
                    nc.vector.scalar_tensor_tensor(
                        uu_g[csl, nn0:nn1], ups[csl, nn0:nn1], 1.0,
                        sg[csl, nn0:nn1], op0=Alu.mult, op1=Alu.mult)
            uu = inp.tile([64, PIX], f32, tag="uu")
            for g in range(2):
                for hl in range(2):
                    nc.sync.dma_start(
                        uu[32 * hl + 16 * g:32 * hl + 16 * g + 16, :],
                        uu_g[32 * g + 16 * hl:32 * g + 16 * hl + 16, :])

            # ---------- gated gutter buffer ----------
            gated = inp.tile([64, GBUF], f32, tag="gated")
            nc.gpsimd.memset(gated[:], 0.0)

            # dedicated double-buffers for scatter-read tiles (rows beyond the
            # written band must be initialized: CoreSim + data hygiene)
            eAB = [inp.tile([128, PATCH], f32, tag=f"E{i}", name=f"E{i}") for i in range(2)]
            gAB = [inp.tile([128, WS2], f32, tag=f"gr{i}", name=f"gr{i}") for i in range(2)]
            for t_ in eAB + gAB:
                nc.gpsimd.memset(t_[:], 0.0)

            # ---------- attention bands ----------
            for b in range(NB):
                bp = _band_pix(b)
                nrows = 4 if b < 7 else 2
                for hl in range(2):
                    hsl = slice(32 * hl, 32 * hl + 32)
                    osl = slice(32 * hl, 32 * hl + 1)
                    # rel + brel -> exp -> G scatter
                    rel_ps = ps1.tile([128, 512], f32, tag="relps")
                    nc.tensor.matmul(rel_ps[0:bp, 0:WS2],
                                     f_vpad[hsl, 240 + 120 * b:240 + 120 * b + bp],
                                     wrelT[hsl, :], start=True, stop=False)
                    nc.tensor.matmul(rel_ps[0:bp, 0:WS2], ones2[osl, 0:bp],
                                     brel[osl, :], start=False, stop=True)
                    grel = gAB[(2 * b + hl) % 2]
                    nc.scalar.activation(grel[0:bp, :], rel_ps[0:bp, 0:WS2], Act.Exp)
                    G = bandp.tile([128, PATCH], f32, tag="G")
                    nc.gpsimd.local_scatter(
                        G.bitcast(dt.uint16)[:, :],
                        grel.bitcast(dt.uint16)[:, :],
                        gidx[:], channels=128, num_elems=2 * PATCH,
                        num_idxs=2 * WS2)
                    top = 30 * max(0, 8 - 4 * b)
                    bot = 30 * min(20, 38 - 4 * b)
                    if top > 0:
                        nc.vector.memset(G[:, 0:top], 0.0)
                    if bot < PATCH:
                        nc.vector.memset(G[:, bot:PATCH], 0.0)
                    # qk patch matmul (pre-scaled by 1/T via f_scaled)
                    qk_psA = ps1.tile([128, 512], f32, tag="qkpsA")
                    qk_psB = ps1.tile([128, 512], f32, tag="qkpsB")
                    for (nn0, nn1), qk_ps in (((0, 512), qk_psA),
                                              ((512, PATCH), qk_psB)):
                        nc.tensor.matmul(qk_ps[0:bp, 0:nn1 - nn0],
                                         f_scaled[hsl, 120 * b:120 * b + bp],
                                         f_vpad[hsl, 120 * b + nn0:120 * b + nn1],
                                         start=True, stop=True)
                    expqk = bandp.tile([128, PATCH], f32, tag="expqk")
                    nc.scalar.activation(expqk[0:bp, 0:512], qk_psA[0:bp, 0:512],
                                         Act.Exp)
                    nc.scalar.activation(expqk[0:bp, 512:PATCH],
                                         qk_psB[0:bp, 0:PATCH - 512], Act.Exp)
                    E = eAB[(2 * b + hl) % 2]
                    Z = bandp.tile([128, 1], f32, tag="Z")
                    nc.vector.scalar_tensor_tensor(
                        E[0:bp, :], expqk[0:bp, :], 1.0, G[0:bp, :],
                        op0=Alu.mult, op1=Alu.mult, accum_out=Z[0:bp, :])
                    R = bandp.tile([128, 1], f32, tag="R")
                    nc.vector.reciprocal(R[0:bp, :], Z[0:bp, :])
                    nc.vector.tensor_scalar_mul(E[0:bp, :], E[0:bp, :], R[0:bp, :])
                    # compact attention shard out
                    attn_c = bandp.tile([128, 226], f32, tag="attnc")
                    nc.gpsimd.local_scatter(
                        attn_c.bitcast(dt.uint16)[:, :],
                        E.bitcast(dt.uint16)[:, :],
                        exidx[:], channels=128, num_elems=452,
                        num_idxs=2 * PATCH)
                    nc.sync.dma_start(d_attn[hl, 120 * b:120 * b + bp, :],
                                      attn_c[0:bp, 0:WS2])
                    # transposes + aggregation
                    agg_ps = ps1.tile([64, 512], f32, tag="aggps")
                    valid = [tt for tt in range(b - 2, b + 3) if 0 <= tt < NB]
                    for tt in valid:
                        j = tt - (b - 2)
                        et_ps = psT.tile([128, 512], f32, tag="etps")
                        nc.tensor.transpose(et_ps[0:120, 0:bp],
                                            E[0:bp, 120 * j:120 * j + 120],
                                            ident[0:bp, 0:bp])
                        et = bandp.tile([128, 120], f32, tag="et")
                        nc.any.tensor_copy(et[0:120, 0:bp], et_ps[0:120, 0:bp])
                        vch = vT[0:120, tt, :].rearrange("p (hl c) -> p hl c", hl=2)
                        nc.tensor.matmul(agg_ps[hsl, 0:bp], vch[:, hl, :],
                                         et[0:120, 0:bp],
                                         start=(tt == valid[0]),
                                         stop=(tt == valid[-1]))
                    # gating into gutter layout
                    gbase = (4 * b + 2) * GUT + 2
                    gv = gated[hsl, gbase:gbase + GUT * nrows].rearrange(
                        "c (r x) -> c r x", x=GUT)[:, :, 0:30]
                    nc.vector.scalar_tensor_tensor(
                        gv,
                        agg_ps[hsl, 0:bp].rearrange("c (r x) -> c r x", x=30),
                        1.0,
                        uu[hsl, 120 * b:120 * b + bp].rearrange(
                            "c (r x) -> c r x", x=30),
                        op0=Alu.mult, op1=Alu.mult)

            # ---------- dwconv (25 diagonal matmuls) + projection ----------
            dw_sb = inp.tile([64, PIX], f32, tag="dwsb")
            for ci, (r0, r1) in enumerate(((0, 17), (17, 30))):
                ncols = 30 * (r1 - r0)
                dw_ps = ps1.tile([64, 512], f32, tag="ph", name=f"dw_ps{ci}")
                for t in range(25):
                    ky, kx = divmod(t, 5)
                    base = (r0 + ky) * GUT + kx
                    tap = gated[:, base:base + GUT * (r1 - r0)].rearrange(
                        "c (r x) -> c r x", x=GUT)[:, :, 0:30]
                    nc.tensor.matmul(dw_ps[0:64, 0:ncols],
                                     wdw[:, 64 * t:64 * t + 64], tap,
                                     start=(t == 0), stop=(t == 24))
                nc.vector.tensor_copy(dw_sb[:, 30 * r0:30 * r1],
                                      dw_ps[0:64, 0:ncols])
            pj_ps = ps1.tile([128, 1024], f32, tag="ph")
            for nn0, nn1 in ((0, 512), (512, PIX)):
                nc.tensor.matmul(pj_ps[:, nn0:nn1], wproj[:], dw_sb[:, nn0:nn1],
                                 start=True, stop=True)
            pj_sb = inp.tile([128, PIX], f32, tag="pjsb")
            for nn0, nn1 in ((0, 512), (512, PIX)):
                nc.vector.tensor_copy(pj_sb[:, nn0:nn1], pj_ps[:, nn0:nn1])
            nc.sync.dma_start(d_proj[:, :], pj_sb[:])

    nc.finalize()
    return nc


def _prep_inputs(q, v, u, Wqk, bqk, Wv, bv, Wu, bu, Wrel, brel, Wdw, Wproj):
    """Per-core input maps. Core k: n = k//4, heads (2*(k%4), 2*(k%4)+1)."""
    g_idx, ex_idx = _build_indices()
    ident = np.eye(128, dtype=np.float32)
    maps = []
    for k in range(8):
        n, hp = k // 4, k % 4
        h0 = 2 * hp
        heads = (h0, h0 + 1)
        # qk_feat rows for these heads: [64*hp, 64*hp+64)
        wqk_sl = Wqk[64 * hp:64 * hp + 64]           # (64, 256)
        wqkT = np.stack([wqk_sl[:, 0:128].T, wqk_sl[:, 128:256].T])  # (2,128,64)
        bqk_sl = bqk[64 * hp:64 * hp + 64].reshape(64, 1)
        wrelT = np.concatenate([Wrel[h].T for h in heads], 0)        # (64, 225)
        brel2 = np.zeros((33, WS2), np.float32)
        brel2[0] = brel[h0]
        brel2[32] = brel[h0 + 1]
        # conv weight rows for head h, group g: row (h*16 + cc) of group g
        # col order within a g-block: (hl, cc)
        rows = lambda h: slice(h * 16, h * 16 + 16)
        wvT = np.stack([np.concatenate(
            [Wv[g][rows(h)] for h in heads], 0).T for g in range(2)])  # (2,128,32)
        wuT = np.stack([np.concatenate(
            [Wu[g][rows(h)] for h in heads], 0).T for g in range(2)])
        # biases: vv channel (g, h*16+cc) -> local (g, hl, cc)
        bvl = np.stack([np.concatenate(
            [bv[128 * g + 16 * h:128 * g + 16 * h + 16] for h in heads])
            for g in range(2)])[None]                                # (1,2,32)
        bul = np.stack([np.concatenate(
            [bu[128 * g + 16 * h:128 * g + 16 * h + 16] for h in heads])
            for g in range(2)])[None]
        # local channel order (head-major): c = hl*32 + g*16 + cc
        # global channel for dw/proj: h*32 + g*16 + cc  (h = h0 + hl)
        gch = np.array([(h0 + hl) * 32 + g * 16 + cc
                        for hl in range(2) for g in range(2)
                        for cc in range(16)])
        wdw_k = Wdw[gch, 0]                                          # (64, 5, 5)
        wdw_diag = np.zeros((64, 25 * 64), np.float32)
        for t in range(25):
            ky, kx = divmod(t, 5)
            wdw_diag[np.arange(64), 64 * t + np.arange(64)] = wdw_k[:, ky, kx]
        wprojT = Wproj[:, gch].T.copy()                              # (64, 128)
        maps.append({
            "q": np.ascontiguousarray(q[n].reshape(DQK, PIX)),
            "v": np.ascontiguousarray(v[n].reshape(DVU, PIX)),
            "u": np.ascontiguousarray(u[n].reshape(DVU, PIX)),
            "wqkT": np.ascontiguousarray(wqkT),
            "bqk": np.ascontiguousarray(bqk_sl),
            "wrelT": np.ascontiguousarray(wrelT),
            "brel": brel2,
            "wvT": np.ascontiguousarray(wvT.astype(np.float32)),
            "bv": np.ascontiguousarray(bvl.astype(np.float32)),
            "wuT": np.ascontiguousarray(wuT.astype(np.float32)),
            "bu": np.ascontiguousarray(bul.astype(np.float32)),
            "wdw": wdw_diag,
            "wprojT": np.ascontiguousarray(wprojT),
            "g_idx": g_idx,
            "ex_idx": ex_idx,
            "ident": ident,
        })
    return maps


def kernel(q, k, v, u, Wqk, bqk, Wv, bv, Wu, bu, Wrel, brel, Wdw, Wproj, bproj):
    """Full inputs in, full outputs out. k is unused (overwritten by
    linear_QK(q) in the reference)."""
    from concourse.bass_utils import run_bass_kernel_spmd

    q = np.asarray(q, np.float32)
    v = np.asarray(v, np.float32)
    u = np.asarray(u, np.float32)
    if "nc" not in _CACHE:
        _CACHE["nc"] = _build_program()
    nc = _CACHE["nc"]
    in_maps = _prep_inputs(q, v, u,
                           np.asarray(Wqk, np.float32), np.asarray(bqk, np.float32),
                           np.asarray(Wv, np.float32), np.asarray(bv, np.float32),
                           np.asarray(Wu, np.float32), np.asarray(bu, np.float32),
                           np.asarray(Wrel, np.float32), np.asarray(brel, np.float32),
                           np.asarray(Wdw, np.float32), np.asarray(Wproj, np.float32))
    res = run_bass_kernel_spmd(nc, in_maps, core_ids=list(range(8))).results

    local_attn = np.empty((N, NH, WS2, PIX), np.float32)
    out = np.zeros((PIX, N, DOUT), np.float32)
    for kk in range(8):
        n, hp = kk // 4, kk % 4
        attn = res[kk]["attn_out"]          # (2, 900, 225)
        local_attn[n, 2 * hp] = attn[0].T
        local_attn[n, 2 * hp + 1] = attn[1].T
        out[:, n, :] += res[kk]["proj_out"].T   # (900, 128)
    out += np.asarray(bproj, np.float32)[None, None, :]
    return out, local_attn
